# revision 3
# baseline (speedup 1.0000x reference)
"""Fused linear + cross-entropy loss (chunked logsumexp) on 8 NeuronCores.

Strategy: tensor-parallel over vocab. Each core holds a 4000-row shard of
head_weight (padded to 4096), computes logits = h @ W_c^T for all 4096
tokens via bf16 matmuls (everything SBUF-resident), and reduces
sum(exp(logit)) per token on the ACT engine (exp + accum_out). The
target-logit term is a per-token dot h[t] . W[label_t] computed on the
DVE from host-gathered rows (data-parallel over tokens). The host does
only glue: transpose/cast/shard, the final log over 4096 values, and the
weighted mean.
"""

import numpy as np
import ml_dtypes

T = 4096
D = 1024
V = 32000
NCORES = 8
VSH = V // NCORES        # 4000 vocab rows per core
VPAD = 4096              # padded so 8 chunks of 512 (zero cols -> exp(0)=1)
NCHUNK = 512             # matmul free dim / one PSUM bank
TT = T // 128            # 32 token tiles
TB = 4                   # token tiles per ht block
TLOC = T // NCORES       # 512 tokens per core for the target dot
JT = TLOC // 128         # 4 local token tiles

_CACHE = {}


def _build(kt, t=T, vpad=VPAD, jt=JT, d=D, do_compile=True):
    """Build+compile the SPMD Bass program. kt = number of 128-deep k
    tiles (8, or 9 when a nonzero head_bias is folded in as an extra
    contraction row)."""
    import concourse.bass as bass
    import concourse.mybir as mybir
    import concourse.tile as tile
    from concourse import bacc

    f32 = mybir.dt.float32
    bf16 = mybir.dt.bfloat16
    AF = mybir.ActivationFunctionType
    ALU = mybir.AluOpType

    tt = t // 128            # token tiles
    ntb = max(tt // TB, 1)   # ht col blocks
    tb = tt // ntb           # token tiles per block
    nch = vpad // 2 // NCHUNK  # psum banks per half

    nc = bacc.Bacc("TRN2", target_bir_lowering=False, debug=False)

    hT_d = nc.dram_tensor("ht", [kt, 128, t], bf16, kind="ExternalInput")
    wT_d = nc.dram_tensor("wt", [kt, 128, vpad], bf16, kind="ExternalInput")
    hrow_d = nc.dram_tensor("hrow", [jt, 128, d], bf16, kind="ExternalInput")
    wg_d = nc.dram_tensor("wg", [jt, 128, d], bf16, kind="ExternalInput")
    sums_d = nc.dram_tensor("sums", [128, tt], f32, kind="ExternalOutput")
    tgt_d = nc.dram_tensor("tgt", [128, jt], f32, kind="ExternalOutput")

    with tile.TileContext(nc) as tc:
        with (
            tc.tile_pool(name="w", bufs=1) as wpool,
            tc.tile_pool(name="h", bufs=1) as hpool,
            tc.tile_pool(name="dot", bufs=1) as dpool,
            tc.tile_pool(name="stat", bufs=1) as spool,
            tc.tile_pool(name="sink", bufs=2) as kpool,
            tc.tile_pool(name="ps", bufs=2, space="PSUM") as ppool,
        ):
            # Resident weight shard, split per (k, half) for early start.
            wt = [[None] * 2 for _ in range(kt)]
            for half in range(2):
                for k in range(kt):
                    tl = wpool.tile([128, vpad // 2], bf16, tag=f"w{k}_{half}")
                    nc.sync.dma_start(
                        tl[:],
                        wT_d[k, :, half * (vpad // 2):(half + 1) * (vpad // 2)],
                    )
                    wt[k][half] = tl
            # Resident hidden (transposed), split in ntb col blocks.
            ht = [[None] * ntb for _ in range(kt)]
            for b in range(ntb):
                for k in range(kt):
                    tl = hpool.tile([128, tb * 128], bf16, tag=f"h{k}_{b}")
                    nc.sync.dma_start(
                        tl[:], hT_d[k, :, b * tb * 128:(b + 1) * tb * 128]
                    )
                    ht[k][b] = tl

            # Target dot: tgt[p, j] = sum_d hrow[j,p,d] * wg[j,p,d]  (DVE)
            tgt_sb = spool.tile([128, jt], f32, tag="tgt")
            for j in range(jt):
                hr = dpool.tile([128, d], bf16, tag=f"hr{j}")
                wr = dpool.tile([128, d], bf16, tag=f"wr{j}")
                nc.sync.dma_start(hr[:], hrow_d[j])
                nc.sync.dma_start(wr[:], wg_d[j])
                dsink = kpool.tile([128, d], f32, tag="dsink")
                nc.vector.tensor_tensor(dsink[:], hr[:], wr[:], ALU.mult)
                nc.vector.tensor_reduce(
                    tgt_sb[:, j:j + 1],
                    dsink[:],
                    axis=mybir.AxisListType.X,
                    op=ALU.add,
                )
            nc.sync.dma_start(tgt_d[:], tgt_sb[:])

            # Main loop: logits for 128 tokens x (vpad/2) vocab per step,
            # then exp+rowsum on ACT. Two PSUM buffers ping-pong.
            hsums = spool.tile([128, tt, 2], f32, tag="hsums")
            for half in range(2):
                for t_i in range(tt):
                    ps = ppool.tile([128, nch, NCHUNK], f32, tag="ps")
                    for k in range(kt):
                        lhsT = ht[k][t_i // tb][
                            :, (t_i % tb) * 128:(t_i % tb + 1) * 128
                        ]
                        for c in range(nch):
                            nc.tensor.matmul(
                                ps[:, c, :],
                                lhsT,
                                wt[k][half][:, c * NCHUNK:(c + 1) * NCHUNK],
                                start=(k == 0),
                                stop=(k == kt - 1),
                            )
                    esink = kpool.tile([128, nch * NCHUNK], bf16, tag="esink")
                    nc.scalar.activation(
                        esink[:],
                        ps[:, :, :],
                        AF.Exp,
                        accum_out=hsums[:, t_i, half:half + 1],
                    )

            sums_sb = spool.tile([128, tt], f32, tag="sums")
            nc.vector.tensor_tensor(
                sums_sb[:], hsums[:, :, 0], hsums[:, :, 1], ALU.add
            )
            nc.sync.dma_start(sums_d[:], sums_sb[:])

    if do_compile:
        nc.compile()
    return nc, (hT_d, wT_d, hrow_d, wg_d, sums_d, tgt_d)


def _get_nc(kt):
    if kt not in _CACHE:
        _CACHE[kt] = _build(kt)[0]
    return _CACHE[kt]


def kernel(hidden_states, head_weight, head_bias, labels, loss_weight):
    from concourse.bass_utils import run_bass_kernel_spmd

    bf16 = ml_dtypes.bfloat16
    h = np.ascontiguousarray(np.asarray(hidden_states, dtype=np.float32))
    W = np.ascontiguousarray(np.asarray(head_weight, dtype=np.float32))
    b = np.asarray(head_bias, dtype=np.float32)
    lab = np.asarray(labels).astype(np.int64)
    lw = np.asarray(loss_weight, dtype=np.float32)

    use_bias = bool(np.any(b))
    kt = 9 if use_bias else 8
    nc = _get_nc(kt)

    # Shared across cores: hidden transposed, k-tiled, bf16.
    hT_k = np.zeros((kt, 128, T), dtype=bf16)
    hT_k[:8] = np.ascontiguousarray(h.T).reshape(8, 128, T).astype(bf16)
    if use_bias:
        hT_k[8, 0, :] = bf16(1.0)

    Wg = W[lab]                     # [T, D] gathered target rows
    tgt_bias = b[lab]               # [T]

    in_maps = []
    for c in range(NCORES):
        Wc = W[c * VSH:(c + 1) * VSH]                    # [4000, 1024]
        wT = np.zeros((kt, 128, VPAD), dtype=bf16)
        wT[:8, :, :VSH] = (
            np.ascontiguousarray(Wc.T).reshape(8, 128, VSH).astype(bf16)
        )
        if use_bias:
            wT[8, 0, :VSH] = b[c * VSH:(c + 1) * VSH].astype(bf16)
        hrow = h[c * TLOC:(c + 1) * TLOC].reshape(JT, 128, D).astype(bf16)
        wg = Wg[c * TLOC:(c + 1) * TLOC].reshape(JT, 128, D).astype(bf16)
        in_maps.append({"ht": hT_k, "wt": wT, "hrow": hrow, "wg": wg})

    res = run_bass_kernel_spmd(nc, in_maps, core_ids=list(range(NCORES)))

    # Combine: sums[c][p, t] is sum(exp(logits)) over core c's vocab shard
    # for token t*128+p, plus (VPAD-VSH) padding columns contributing
    # exp(0)=1 each.
    S = np.stack([r["sums"] for r in res.results])          # [8, 128, TT]
    sumexp = S.transpose(0, 2, 1).reshape(NCORES, T).astype(np.float64)
    sumexp -= float(VPAD - VSH)
    logz = np.log(sumexp.sum(axis=0))                       # [T]

    G = np.stack([r["tgt"] for r in res.results])           # [8, 128, JT]
    tgt = G.transpose(0, 2, 1).reshape(T) + tgt_bias        # [T]

    nll = logz - tgt
    lw64 = lw.astype(np.float64)
    loss = (lw64 * nll).sum() / lw64.sum()
    return np.float32(loss)


# revision 4
# speedup vs baseline: 1.9525x; 1.9525x over previous
"""Fused linear + cross-entropy loss (chunked logsumexp) on 8 NeuronCores.

Strategy: tensor-parallel over vocab. Each core holds a 4000-row shard of
head_weight (padded to 4096), computes logits = h @ W_c^T for all 4096
tokens (fp8e4m3 DoubleRow matmuls by default; bf16 fallback), and reduces
sum(exp(logit)) per token on the ACT engine (exp with accum_out; the
pre-exp rescale for the fp8 weight scaling rides the ACT's free scale
operand). The target-logit term is a per-token dot h[t] . W[label_t]
computed on the DVE in bf16 from host-gathered rows (data-parallel over
tokens). The host does only glue: transpose/cast/shard, the final log
over 4096 values, and the weighted mean.
"""

import numpy as np
import ml_dtypes

T = 4096
D = 1024
V = 32000
NCORES = 8
VSH = V // NCORES        # 4000 vocab rows per core
VPAD = 4096              # padded so 8 chunks of 512 (zero cols -> exp(0)=1)
NCHUNK = 512             # matmul free dim / one PSUM bank
TT = T // 128            # 32 token tiles
TBLK = 512               # tokens per resident ht block
TLOC = T // NCORES       # 512 tokens per core for the target dot
JT = TLOC // 128         # 4 local token tiles

W_SCALE = 32.0           # fp8 path: W is scaled by this before casting

USE_FP8 = True

_CACHE = {}


def _build(kt, mode, t=T, vpad=VPAD, jt=JT, d=D, do_compile=True):
    """Build+compile the SPMD Bass program.

    kt: number of 128-deep k tiles (8, or 9 when a nonzero head_bias is
        folded in as an extra contraction row).
    mode: "bf16" (plain matmuls) or "fp8dr" (fp8e4m3 DoubleRow, kt must
        be even).
    """
    import concourse.bass as bass
    import concourse.mybir as mybir
    import concourse.tile as tile
    from concourse import bacc

    f32 = mybir.dt.float32
    bf16 = mybir.dt.bfloat16
    fp8 = mybir.dt.float8e4
    AF = mybir.ActivationFunctionType
    ALU = mybir.AluOpType

    fp8dr = mode == "fp8dr"
    mdt = fp8 if fp8dr else bf16
    act_scale = (1.0 / W_SCALE) if fp8dr else 1.0
    if fp8dr:
        assert kt % 2 == 0

    tt = t // 128              # token tiles
    tb = min(TBLK // 128, tt)  # token tiles per ht block
    ntb = tt // tb             # ht col blocks
    nch = vpad // 2 // NCHUNK  # psum banks per half
    nwb = 2                    # weight col blocks per half

    nc = bacc.Bacc("TRN2", target_bir_lowering=False, debug=False)

    # DRAM layouts put the partition dim first and the k-tile dim in the
    # middle so DoubleRow can pair adjacent k sub-tiles.
    hT_d = nc.dram_tensor("ht", [128, kt, t], mdt, kind="ExternalInput")
    wT_d = nc.dram_tensor("wt", [128, kt, vpad], mdt, kind="ExternalInput")
    hrow_d = nc.dram_tensor("hrow", [jt, 128, d], bf16, kind="ExternalInput")
    wg_d = nc.dram_tensor("wg", [jt, 128, d], bf16, kind="ExternalInput")
    hsums_d = nc.dram_tensor("hsums", [128, tt, 2], f32, kind="ExternalOutput")
    tgt_d = nc.dram_tensor("tgt", [128, jt], f32, kind="ExternalOutput")

    with tile.TileContext(nc) as tc:
        with (
            tc.tile_pool(name="w", bufs=1) as wpool,
            tc.tile_pool(name="h", bufs=1) as hpool,
            tc.tile_pool(name="dot", bufs=1) as dpool,
            tc.tile_pool(name="stat", bufs=1) as spool,
            tc.tile_pool(name="sink", bufs=2) as kpool,
            tc.tile_pool(name="ps", bufs=2, space="PSUM") as ppool,
        ):
            wcols = vpad // 2 // nwb
            wt = [[None] * (2 * nwb) for _ in range(1)][0]
            wt = [None] * (2 * nwb)       # [half*nwb + b] -> [128, kt, wcols]
            ht = [None] * ntb             # [b] -> [128, kt, tb*128]

            def load_w(half, b):
                tl = wpool.tile([128, kt, wcols], mdt, tag=f"w{half}_{b}")
                lo = (half * nwb + b) * wcols
                nc.sync.dma_start(tl[:], wT_d[:, :, lo:lo + wcols])
                wt[half * nwb + b] = tl

            def load_h(b):
                tl = hpool.tile([128, kt, tb * 128], mdt, tag=f"h{b}")
                lo = b * tb * 128
                nc.sync.dma_start(tl[:], hT_d[:, :, lo:lo + tb * 128])
                ht[b] = tl

            # First-needed data first: w half0 block0, h block0, then the
            # rest; compute can start as soon as the first pieces land.
            load_w(0, 0)
            load_h(0)
            load_w(0, 1)
            for b in range(1, ntb):
                load_h(b)
            for b in range(nwb):
                load_w(1, b)

            # Target dot: tgt[p, j] = sum_d hrow[j,p,d] * wg[j,p,d]  (DVE)
            tgt_sb = spool.tile([128, jt], f32, tag="tgt")
            for j in range(jt):
                hr = dpool.tile([128, d], bf16, tag=f"hr{j}")
                wr = dpool.tile([128, d], bf16, tag=f"wr{j}")
                nc.sync.dma_start(hr[:], hrow_d[j])
                nc.sync.dma_start(wr[:], wg_d[j])
                dsink = kpool.tile([128, d], f32, tag="dsink")
                nc.vector.tensor_tensor(dsink[:], hr[:], wr[:], ALU.mult)
                nc.vector.tensor_reduce(
                    tgt_sb[:, j:j + 1],
                    dsink[:],
                    axis=mybir.AxisListType.X,
                    op=ALU.add,
                )
            nc.sync.dma_start(tgt_d[:], tgt_sb[:])

            # Main loop: logits for 128 tokens x (vpad/2) vocab per step,
            # then exp+rowsum on ACT. Two PSUM buffers ping-pong.
            hsums = spool.tile([128, tt, 2], f32, tag="hsums")
            for half in range(2):
                for t_i in range(tt):
                    hblk = ht[t_i // tb]
                    mlo = (t_i % tb) * 128
                    ps = ppool.tile([128, nch, NCHUNK], f32, tag="ps")
                    if fp8dr:
                        for k2 in range(kt // 2):
                            lhsT = hblk[:, 2 * k2:2 * k2 + 2, mlo:mlo + 128]
                            for c in range(nch):
                                wb = wt[half * nwb + (c * NCHUNK) // wcols]
                                co = (c * NCHUNK) % wcols
                                nc.tensor.matmul(
                                    ps[:, c, :],
                                    lhsT,
                                    wb[:, 2 * k2:2 * k2 + 2, co:co + NCHUNK],
                                    start=(k2 == 0),
                                    stop=(k2 == kt // 2 - 1),
                                    perf_mode=mybir.MatmulPerfMode.DoubleRow,
                                )
                    else:
                        for k in range(kt):
                            lhsT = hblk[:, k, mlo:mlo + 128]
                            for c in range(nch):
                                wb = wt[half * nwb + (c * NCHUNK) // wcols]
                                co = (c * NCHUNK) % wcols
                                nc.tensor.matmul(
                                    ps[:, c, :],
                                    lhsT,
                                    wb[:, k, co:co + NCHUNK],
                                    start=(k == 0),
                                    stop=(k == kt - 1),
                                )
                    esink = kpool.tile([128, nch * NCHUNK], bf16, tag="esink")
                    nc.scalar.activation(
                        esink[:],
                        ps[:, :, :],
                        AF.Exp,
                        scale=act_scale,
                        accum_out=hsums[:, t_i, half:half + 1],
                    )

            nc.sync.dma_start(hsums_d[:], hsums[:])

    if do_compile:
        nc.compile()
    return nc


def _get_nc(kt, mode):
    key = (kt, mode)
    if key not in _CACHE:
        _CACHE[key] = _build(kt, mode)
    return _CACHE[key]


def kernel(hidden_states, head_weight, head_bias, labels, loss_weight):
    from concourse.bass_utils import run_bass_kernel_spmd

    bf16 = ml_dtypes.bfloat16
    fp8 = ml_dtypes.float8_e4m3
    h = np.ascontiguousarray(np.asarray(hidden_states, dtype=np.float32))
    W = np.ascontiguousarray(np.asarray(head_weight, dtype=np.float32))
    b = np.asarray(head_bias, dtype=np.float32)
    lab = np.asarray(labels).astype(np.int64)
    lw = np.asarray(loss_weight, dtype=np.float32)

    use_bias = bool(np.any(b))
    mode = "fp8dr" if (USE_FP8 and not use_bias) else "bf16"
    mdt = fp8 if mode == "fp8dr" else bf16
    wscale = W_SCALE if mode == "fp8dr" else 1.0
    kt = 9 if use_bias else 8
    nc = _get_nc(kt, mode)

    # Shared across cores: hidden transposed into [128, kt, T]:
    # ht[p, k, t] = h[t, k*128 + p].
    hT_k = np.zeros((128, kt, T), dtype=mdt)
    hT_k[:, :8, :] = (
        np.ascontiguousarray(h.T).reshape(8, 128, T).transpose(1, 0, 2)
    ).astype(mdt)
    if use_bias:
        hT_k[0, 8, :] = mdt(1.0)

    Wg = W[lab]                     # [T, D] gathered target rows
    tgt_bias = b[lab]               # [T]

    in_maps = []
    for c in range(NCORES):
        Wc = W[c * VSH:(c + 1) * VSH]                    # [4000, 1024]
        wT = np.zeros((128, kt, VPAD), dtype=mdt)
        wT[:, :8, :VSH] = (
            (np.ascontiguousarray(Wc.T) * wscale)
            .reshape(8, 128, VSH)
            .transpose(1, 0, 2)
            .astype(mdt)
        )
        if use_bias:
            wT[0, 8, :VSH] = b[c * VSH:(c + 1) * VSH].astype(mdt)
        hrow = h[c * TLOC:(c + 1) * TLOC].reshape(JT, 128, D).astype(bf16)
        wg = Wg[c * TLOC:(c + 1) * TLOC].reshape(JT, 128, D).astype(bf16)
        in_maps.append({"ht": hT_k, "wt": wT, "hrow": hrow, "wg": wg})

    res = run_bass_kernel_spmd(nc, in_maps, core_ids=list(range(NCORES)))

    # Combine. hsums[c][p, t, j] is sum(exp(logits)) over a half of core
    # c's vocab shard for token t*128+p; padding columns contribute
    # exp(0)=1 each (they live in the second half).
    S = np.stack([r["hsums"] for r in res.results])         # [8, 128, TT, 2]
    S = S.sum(axis=3)                                       # [8, 128, TT]
    sumexp = S.transpose(0, 2, 1).reshape(NCORES, T).astype(np.float64)
    sumexp -= float(VPAD - VSH)
    logz = np.log(sumexp.sum(axis=0))                       # [T]

    G = np.stack([r["tgt"] for r in res.results])           # [8, 128, JT]
    tgt = G.transpose(0, 2, 1).reshape(T) + tgt_bias        # [T]

    nll = logz - tgt
    lw64 = lw.astype(np.float64)
    loss = (lw64 * nll).sum() / lw64.sum()
    return np.float32(loss)
